# revision 1
# baseline (speedup 1.0000x reference)
import numpy as np

WIN = 12  # max_relative_position
B, L, H, E = 4, 1024, 8, 64
SCALE = 1.0 / float(np.sqrt(E))


def _rel_embed(jnp, table, length):
    r = jnp.arange(length)
    idx = jnp.clip(r[None, :] - r[:, None], -WIN, WIN) + WIN  # [L, L]
    return table[idx]  # [L, L, E]


def _make_shard_fn(jnp):
    def shard_fn(x1, y1, vx1, vy1, tk, tvx, tvy):
        # x1,y1,vx1,vy1: [B, L, E] (one head); tables: [2*WIN+1, E]
        rel_k = _rel_embed(jnp, tk, L)
        rel_vy = _rel_embed(jnp, tvy, L)
        rel_vx = _rel_embed(jnp, tvx, L)
        s1 = jnp.einsum('ble,bse->bls', x1, y1) + jnp.einsum('ble,lse->bls', x1, rel_k)
        import jax
        a1 = jax.nn.softmax(SCALE * s1, axis=-1)
        a2 = jax.nn.softmax(SCALE * jnp.swapaxes(s1, -2, -1), axis=-1)
        vy = jnp.einsum('bls,bsd->bld', a1, vy1) + jnp.einsum('bls,lsd->bld', a1, rel_vy)
        vx = jnp.einsum('bls,bsd->bld', a2, vx1) + jnp.einsum('bls,lsd->bld', a2, rel_vx)
        return vy, vx

    return shard_fn


def kernel(x, y, v_x, v_y, rel_k_table, rel_vx_table, rel_vy_table,
           attn_mask1=None, attn_mask2=None):
    import jax
    import jax.numpy as jnp

    x = np.asarray(x, np.float32)
    y = np.asarray(y, np.float32)
    v_x = np.asarray(v_x, np.float32)
    v_y = np.asarray(v_y, np.float32)
    tk = np.asarray(rel_k_table, np.float32)
    tvx = np.asarray(rel_vx_table, np.float32)
    tvy = np.asarray(rel_vy_table, np.float32)

    # head-parallel shards: [H, B, L, E]
    xh = np.ascontiguousarray(np.moveaxis(x, 2, 0))
    yh = np.ascontiguousarray(np.moveaxis(y, 2, 0))
    vxh = np.ascontiguousarray(np.moveaxis(v_x, 2, 0))
    vyh = np.ascontiguousarray(np.moveaxis(v_y, 2, 0))

    shard_fn = _make_shard_fn(jnp)
    vy_h = vx_h = None
    try:
        if len(jax.devices()) >= H:
            pfn = jax.pmap(shard_fn, in_axes=(0, 0, 0, 0, None, None, None))
            vy_h, vx_h = pfn(xh, yh, vxh, vyh, tk, tvx, tvy)
            vy_h = np.asarray(vy_h)
            vx_h = np.asarray(vx_h)
    except Exception:
        vy_h = vx_h = None

    if vy_h is None:
        with jax.default_device(jax.local_devices(backend='cpu')[0]):
            outs = [shard_fn(xh[h], yh[h], vxh[h], vyh[h], tk, tvx, tvy)
                    for h in range(H)]
        vy_h = np.stack([np.asarray(o[0]) for o in outs])
        vx_h = np.stack([np.asarray(o[1]) for o in outs])

    out1 = np.ascontiguousarray(np.moveaxis(vy_h, 0, 2)).astype(np.float32)  # [B,L,H,E]
    out2 = np.ascontiguousarray(np.moveaxis(vx_h, 0, 2)).astype(np.float32)
    return out1, out2

